# revision 16
# baseline (speedup 1.0000x reference)
"""DiagLinear Trainium2 kernel: out[b, j] = x[b, j] * diag(W)[j] + bias[j].

Full-input contract: kernel(x[16384,4096], weight[4096,4096], bias[4096])
-> out[16384,4096], all float32. Internally shards x row-blocks across 8
NeuronCores (data parallel); the weight's diagonal and the bias are
replicated to every core. The per-core Bass/Tile program streams x tiles
[128, 4096] through SBUF: HWDGE load -> DVE mul by broadcast diag -> DVE
add broadcast bias -> HWDGE store.
"""

import numpy as np

try:
    import concourse.bass as bass  # noqa: F401
except ImportError:  # fresh grading dir without the default PYTHONPATH
    import sys

    for p in ("/root/.axon_site", "/root/.axon_site/_ro/trn_rl_repo",
              "/root/.axon_site/_ro/pypackages", "/opt/trn_rl_repo"):
        if p not in sys.path:
            sys.path.append(p)
    import concourse.bass as bass  # noqa: F401

import jax
import jax.numpy as jnp
from jax.experimental.shard_map import shard_map
from jax.sharding import Mesh, PartitionSpec

import concourse.bacc as bacc
import concourse.tile as tile
from concourse import mybir
from concourse.bass2jax import (
    _bass_exec_p,
    install_neuronx_cc_hook,
    partition_id_tensor,
)

N = 4096          # feature dim
B = 16384         # batch dim
NCORES = 8
BS = B // NCORES  # 2048 rows per core
P = 128           # SBUF partitions
NT = BS // P      # 16 tiles of [128, N] per core

IN_NAMES = ("x", "db")
OUT_NAMES = ("y",)
IN_SHAPES = {"x": (BS, N), "db": (2, N)}
OUT_SHAPES = {"y": (BS, N)}


def _build_program(npasses=1):
    """npasses > 1 repeats the whole streaming loop inside one NEFF (the
    timing harness differences two npasses values to cancel dispatch
    overhead; each pass recomputes the same y, so results are unchanged).
    """
    FP = mybir.dt.float32
    # Bacc (not raw Bass): its finalize() runs generate_event_semaphores,
    # which splits multi-wait sync conditions onto EventSemaphore carriers
    # (walrus caps every non-EventSemaphore instruction at 1 sync wait).
    nc = bacc.Bacc()
    x = nc.declare_dram_parameter("x", [BS, N], FP, isOutput=False)
    db = nc.declare_dram_parameter("db", [2, N], FP, isOutput=False)
    y = nc.declare_dram_parameter("y", [BS, N], FP, isOutput=True)

    xv = x[:].rearrange("(n p) m -> n p m", p=P)
    yv = y[:].rearrange("(n p) m -> n p m", p=P)

    with tile.TileContext(nc) as tc:
        with (
            tc.tile_pool(name="const", bufs=1) as cpool,
            tc.tile_pool(name="ipool", bufs=5) as ipool,
            tc.tile_pool(name="opool", bufs=5) as opool,
        ):
            # One DMA broadcasts [diag; bias] to all 128 partitions; the
            # dummy copy makes the DVE observe that DMA's semaphore up
            # front. Walrus rejects TensorTensor instructions with >1
            # sync wait, so the loop is structured so each TT carries at
            # most one: mul is in-place on the input tile (slot's prior
            # reader is the DVE add -> same-engine WAR, no wait; only
            # the x-load RAW wait remains) and add writes a separate
            # output pool (its single wait is the store WAR).
            dbt = cpool.tile([P, 2 * N], FP, tag="db")
            scratch = cpool.tile([P, 1], FP, tag="scratch")
            nc.sync.dma_start(
                dbt[:],
                db[:].rearrange("a n -> (a n)").rearrange("(o f) -> o f", o=1)
                .broadcast_to((P, 2 * N)),
            )
            nc.vector.tensor_copy(scratch[:], dbt[:, 0:1])
            dt = dbt[:, 0:N]
            bt = dbt[:, N:2 * N]
            for _ in range(npasses):
                for i in range(NT):
                    t = ipool.tile([P, N], FP)
                    nc.sync.dma_start(t[:], xv[i, :, :])
                    nc.vector.tensor_mul(t[:], t[:], dt)
                    o = opool.tile([P, N], FP)
                    nc.vector.tensor_add(o[:], t[:], bt)
                    nc.scalar.dma_start(yv[i, :, :], o[:])
    return nc


class _Runner:
    """Caches the Bass program + jitted SPMD executables across calls."""

    def __init__(self):
        install_neuronx_cc_hook()
        devices = jax.devices()[:NCORES]
        assert len(devices) == NCORES
        self.mesh = Mesh(np.asarray(devices), ("core",))
        self._ncs = {}
        self._jits = {}

    def _get_nc(self, npasses):
        if npasses not in self._ncs:
            nc = _build_program(npasses)
            if not nc.is_finalized():
                nc.finalize()
            self._ncs[npasses] = nc
        return self._ncs[npasses]

    def get_jit(self, npasses=1):
        """Jitted fn(x, db, yzero) -> y, running the streaming loop
        `npasses` times inside one NEFF execution (single dispatch)."""
        if npasses in self._jits:
            return self._jits[npasses]

        nc = self._get_nc(npasses)
        pname = nc.partition_id_tensor.name if nc.partition_id_tensor else None
        # bass2jax operand order: params, then donated zero output buffers,
        # then partition id.
        all_in_names = list(IN_NAMES) + list(OUT_NAMES)
        if pname is not None:
            all_in_names.append(pname)

        def _body(x, db, yzero):
            ops = [x, db, yzero]
            if pname is not None:
                ops.append(partition_id_tensor())
            outs = _bass_exec_p.bind(
                *ops,
                out_avals=tuple(
                    jax.core.ShapedArray(OUT_SHAPES[n], np.float32)
                    for n in OUT_NAMES
                ),
                in_names=tuple(all_in_names),
                out_names=tuple(OUT_NAMES),
                lowering_input_output_aliases=(),
                sim_require_finite=True,
                sim_require_nnan=True,
                nc=nc,
            )
            return tuple(outs)

        # No donate_argnums: donation would invalidate device-resident
        # input buffers across repeated timing calls; the kernel writes
        # every element of y, so zero-backed outputs aren't needed.
        fn = jax.jit(
            shard_map(
                _body,
                mesh=self.mesh,
                in_specs=(PartitionSpec("core"),) * 3,
                out_specs=(PartitionSpec("core"),),
                check_rep=False,
            ),
            keep_unused=True,
        )
        self._jits[npasses] = fn
        return fn

    def put_device_inputs(self, x, weight, bias):
        from jax.sharding import NamedSharding

        sh = NamedSharding(self.mesh, PartitionSpec("core"))
        return tuple(
            jax.device_put(a, sh) for a in self.device_inputs(x, weight, bias)
        )

    def device_inputs(self, x, weight, bias):
        """Concatenated global arrays (host). Shard i of axis 0 = core i."""
        db = np.stack(
            [np.ascontiguousarray(np.diagonal(weight)),
             np.ascontiguousarray(bias)]
        ).astype(np.float32, copy=False)                    # (2, 4096)
        xg = np.ascontiguousarray(x).reshape(B, N)
        dbg = np.concatenate([db] * NCORES, axis=0)         # (16, 4096)
        yz = np.zeros((B, N), np.float32)
        return xg, dbg, yz


_RUNNER = None


def _get_runner():
    global _RUNNER
    if _RUNNER is None:
        _RUNNER = _Runner()
    return _RUNNER


def kernel(x, weight, bias):
    r = _get_runner()
    fn = r.get_jit(1)
    xg, dbg, yz = r.device_inputs(x, weight, bias)
    (y,) = fn(xg, dbg, yz)
    return np.asarray(y).reshape(B, N)
